# revision 1
# baseline (speedup 1.0000x reference)
"""Trainium2 Bass kernel for nn_CoAttention.

Data parallel over batch: B=64 split as 8 batches on each of 8 NeuronCores.
Per batch item (Q = x[:384], D = x[384:]):
    QpT = tanh(W @ Q^T + b)          [768, 384]   (PE matmul, ACT tanh+bias)
    LT  = D @ QpT                    [384, 384]   (= L^T, logits)
    ET  = exp(LT - SHIFT), r1 = 1/rowsum(ET)      (ACT exp with accum_out)
    P1T = ET * r1                    (softmax over t, transposed layout)
    E   = ET^T (PE transpose), r2 = 1/rowsum(E)
    P2  = E * r2                     (softmax over s)
    Qp  = QpT^T (PE transpose)
    M1  = P1T^T @ D                  [384, 768]
    Out1 = P2^T @ Qp, Out2 = P2^T @ M1
    out = [Out1 | Out2 | D]          [384, 2304]
Matmul operands are stored as float32r (full-rate PE streaming, values
rounded on write); fp32 is kept for the D passthrough and outputs.
"""

import numpy as np
from contextlib import ExitStack

N_CORES = 8
BPC = 8          # batches per core
H = 768
T = 384
KT = H // 128    # 6
TT = T // 128    # 3
SHIFT = 60.0
MM_DT = "float32r"   # matmul operand storage dtype: float32r | float32 | bfloat16

_cache = {}


def _build_nc():
    import concourse.bass as bass
    import concourse.tile as tile
    from concourse import bacc, mybir
    from concourse.masks import make_identity

    f32 = mybir.dt.float32
    mdt = getattr(mybir.dt, MM_DT)
    AF = mybir.ActivationFunctionType

    nc = bacc.Bacc()
    x_h = nc.declare_dram_parameter("x", [BPC, H, H], f32, isOutput=False)
    xt_h = nc.declare_dram_parameter("xt", [BPC, H, H], f32, isOutput=False)
    wt_h = nc.declare_dram_parameter("wt", [H, H], f32, isOutput=False)
    b_h = nc.declare_dram_parameter("bias", [H], f32, isOutput=False)
    eye_h = nc.declare_dram_parameter("eye", [128, 128], f32, isOutput=False)
    out_h = nc.declare_dram_parameter("out", [BPC, T, 3 * H], f32, isOutput=True)

    with tile.TileContext(nc) as tc, ExitStack() as ctx:
        consts = ctx.enter_context(tc.tile_pool(name="consts", bufs=1))
        xp = ctx.enter_context(tc.tile_pool(name="xp", bufs=2))
        nat = ctx.enter_context(tc.tile_pool(name="nat", bufs=2))
        natq = ctx.enter_context(tc.tile_pool(name="natq", bufs=2))
        trp = ctx.enter_context(tc.tile_pool(name="trp", bufs=2))
        ep = ctx.enter_context(tc.tile_pool(name="ep", bufs=2))
        mop = ctx.enter_context(tc.tile_pool(name="mop", bufs=4))
        smallp = ctx.enter_context(tc.tile_pool(name="small", bufs=2))
        pps = ctx.enter_context(tc.tile_pool(name="pps", bufs=8, space="PSUM"))

        # weights: HWDGE fp32 load + one DVE rounding cast (keeps the SWDGE
        # queue free for the per-batch qt/dt cast loads)
        wt_halves = []
        for wh in range(2):
            wt_tmp = consts.tile([128, KT // 2, H], f32, tag=f"wt_tmp{wh}")
            nc.sync.dma_start(out=wt_tmp, in_=wt_h[wh * 384:(wh + 1) * 384, :]
                              .rearrange("(ki p) o -> p ki o", p=128))
            wt_half = consts.tile([128, KT // 2, H], mdt, tag=f"wt{wh}")
            nc.vector.tensor_copy(wt_half, wt_tmp)
            wt_halves.append(wt_half)
        bias_sb = consts.tile([128, KT], f32)
        nc.sync.dma_start(out=bias_sb, in_=b_h[:].rearrange("(oi p) -> p oi", p=128))
        ident_r = consts.tile([128, 128], mdt)
        nc.gpsimd.dma_start(out=ident_r, in_=eye_h[:, :])
        negshift = consts.tile([128, 1], f32)
        nc.vector.memset(negshift, -SHIFT)

        cp_i = 0

        def copy_out(dst, src):
            # split PSUM->SBUF copies between ACT and DVE (~5/12 to ACT;
            # ACT also does the tanh/exp/identity passes)
            nonlocal cp_i
            if cp_i % 12 in (0, 2, 5, 7, 9):
                nc.scalar.activation(dst, src, AF.Copy)
            else:
                nc.vector.tensor_copy(dst, src)
            cp_i += 1

        for b in range(BPC):
            xb = x_h[b]
            ob = out_h[b].rearrange("(si p) c -> p si c", p=128)

            # ---- QT, DT: direct cast-DMA loads from host-transposed x ----
            xtb = xt_h[b]
            qt = trp.tile([128, KT, T], mdt, tag="qt")
            nc.gpsimd.dma_start(out=qt[:, 0:KT // 2, :],
                                in_=xtb[0:384, 0:T].rearrange("(ki p) t -> p ki t", p=128))
            nc.gpsimd.dma_start(out=qt[:, KT // 2:KT, :],
                                in_=xtb[384:768, 0:T].rearrange("(ki p) t -> p ki t", p=128))
            dt = trp.tile([128, KT, T], mdt, tag="dt")
            nc.gpsimd.dma_start(out=dt, in_=xtb[:, T:H].rearrange("(ki p) t -> p ki t", p=128))
            d_nat = xp.tile([128, TT, H], f32, tag="dnat")
            nc.sync.dma_start(out=d_nat, in_=xb[T:H, :].rearrange("(n p) h -> p n h", p=128))
            # rounded copy of D for use as matmul rhs
            d_mm = nat.tile([128, TT, H], mdt, tag="dmm")
            nc.vector.tensor_copy(d_mm, d_nat)

            # ---- step1: QpT = tanh(W @ Q^T + b) ----
            qpT = trp.tile([128, KT, T], mdt, tag="qpT")
            for oi in range(KT):
                ps = pps.tile([128, T], f32, tag="ps")
                for ki in range(KT):
                    wsb = wt_halves[ki // (KT // 2)]
                    nc.tensor.matmul(ps, wsb[:, ki % (KT // 2), oi * 128:(oi + 1) * 128],
                                     qt[:, ki, :],
                                     start=(ki == 0), stop=(ki == KT - 1))
                nc.scalar.activation(qpT[:, oi, :], ps, AF.Tanh, bias=bias_sb[:, oi:oi + 1])

            # ---- step2: LT = D @ QpT ; ET = exp(LT - SHIFT); r1 = rowsum ----
            eT = ep.tile([128, TT, T], mdt, tag="eT")
            r1 = smallp.tile([128, TT], f32, tag="r1")
            for si in range(TT):
                ps = pps.tile([128, T], f32, tag="ps")
                for ki in range(KT):
                    nc.tensor.matmul(ps, dt[:, ki, si * 128:(si + 1) * 128],
                                     qpT[:, ki, :],
                                     start=(ki == 0), stop=(ki == KT - 1))
                nc.scalar.activation(eT[:, si, :], ps, AF.Exp, bias=negshift[:, 0:1],
                                     accum_out=r1[:, si:si + 1])

            # ---- Qp natural (transpose QpT); runs on PE while ACT does exp ----
            qp = natq.tile([128, TT, H], mdt, tag="qp")
            for ti in range(TT):
                for hf in range(2):
                    ps = pps.tile([128, T], mdt, tag="ps")
                    for j in range(TT):
                        hi = hf * TT + j
                        nc.tensor.transpose(ps[:, j * 128:(j + 1) * 128],
                                            qpT[:, hi, ti * 128:(ti + 1) * 128], ident_r)
                    copy_out(qp[:, ti, hf * T:(hf + 1) * T], ps)

            # ---- E natural = ET^T; r2 = rowsum(E) ----
            e_sb = ep.tile([128, TT, T], mdt, tag="e")
            r2 = smallp.tile([128, TT], f32, tag="r2")
            for ti in range(TT):
                ps = pps.tile([128, T], mdt, tag="ps")
                for si in range(TT):
                    nc.tensor.transpose(ps[:, si * 128:(si + 1) * 128],
                                        eT[:, si, ti * 128:(ti + 1) * 128], ident_r)
                nc.scalar.activation(e_sb[:, ti, :], ps, AF.Identity,
                                     accum_out=r2[:, ti:ti + 1])

            # ---- P1T = ET * r1 ; P2 = E * r2 (in place; after E transposes) ----
            nc.vector.reciprocal(r1, r1)
            for si in range(TT):
                nc.vector.tensor_scalar_mul(eT[:, si, :], eT[:, si, :], r1[:, si:si + 1])
            nc.vector.reciprocal(r2, r2)
            for ti in range(TT):
                nc.vector.tensor_scalar_mul(e_sb[:, ti, :], e_sb[:, ti, :], r2[:, ti:ti + 1])

            # ---- M1 = P1T^T @ D ----
            m1 = mop.tile([128, TT, H], mdt, tag="mo")
            for ti in range(TT):
                for hf in range(2):
                    ps = pps.tile([128, T], f32, tag="ps")
                    for si in range(TT):
                        nc.tensor.matmul(ps, eT[:, si, ti * 128:(ti + 1) * 128],
                                         d_mm[:, si, hf * T:(hf + 1) * T],
                                         start=(si == 0), stop=(si == TT - 1))
                    copy_out(m1[:, ti, hf * T:(hf + 1) * T], ps)

            # ---- Out1 = P2^T @ Qp ; Out2 = P2^T @ M1 ----
            o1 = mop.tile([128, TT, H], f32, tag="mo")
            for si in range(TT):
                for hf in range(2):
                    ps = pps.tile([128, T], f32, tag="ps")
                    for ti in range(TT):
                        nc.tensor.matmul(ps, e_sb[:, ti, si * 128:(si + 1) * 128],
                                         qp[:, ti, hf * T:(hf + 1) * T],
                                         start=(ti == 0), stop=(ti == TT - 1))
                    copy_out(o1[:, si, hf * T:(hf + 1) * T], ps)
            o2 = mop.tile([128, TT, H], f32, tag="mo")
            for si in range(TT):
                for hf in range(2):
                    ps = pps.tile([128, T], f32, tag="ps")
                    for ti in range(TT):
                        nc.tensor.matmul(ps, e_sb[:, ti, si * 128:(si + 1) * 128],
                                         m1[:, ti, hf * T:(hf + 1) * T],
                                         start=(ti == 0), stop=(ti == TT - 1))
                    copy_out(o2[:, si, hf * T:(hf + 1) * T], ps)

            if b == BPC - 1:
                nc.sync.dma_start(out=ob[:, :, 2 * H:3 * H], in_=d_nat)
                for si in range(TT):
                    nc.sync.dma_start(out=ob[:, si:si + 1, 0:H], in_=o1[:, si:si + 1, :])
                for si in range(TT):
                    nc.sync.dma_start(out=ob[:, si:si + 1, H:2 * H], in_=o2[:, si:si + 1, :])
            else:
                nc.sync.dma_start(out=ob[:, :, 0:H], in_=o1)
                nc.sync.dma_start(out=ob[:, :, H:2 * H], in_=o2)
                nc.sync.dma_start(out=ob[:, :, 2 * H:3 * H], in_=d_nat)

    nc.compile()
    return nc


def get_nc():
    if "nc" not in _cache:
        _cache["nc"] = _build_nc()
    return _cache["nc"]


def _prep(x, W, b):
    x = np.ascontiguousarray(np.asarray(x, dtype=np.float32))
    WT = np.ascontiguousarray(np.asarray(W, dtype=np.float32).T)
    bias = np.ascontiguousarray(np.asarray(b, dtype=np.float32))
    xt = np.ascontiguousarray(np.swapaxes(x, 1, 2))
    eye = np.eye(128, dtype=np.float32)
    in_maps = [{"x": x[i * BPC:(i + 1) * BPC], "xt": xt[i * BPC:(i + 1) * BPC],
                "wt": WT, "bias": bias, "eye": eye}
               for i in range(N_CORES)]
    return in_maps


def run(x, W, b, trace=False, tmpdir=None):
    from concourse.bass_utils import run_bass_kernel_spmd
    nc = get_nc()
    res = run_bass_kernel_spmd(nc, _prep(x, W, b), list(range(N_CORES)),
                               trace=trace, tmpdir=tmpdir)
    out = np.concatenate([res.results[i]["out"] for i in range(N_CORES)], axis=0)
    return out, res


def kernel(x, W, b):
    return run(x, W, b)[0]

